# revision 40
# baseline (speedup 1.0000x reference)
"""MiniGRU Trainium2 kernel (v2).

Problem: h_t = (1-z_t) h_{t-1} + z_t g(p_t), with
  z_t = sigmoid(x_t @ Wz^T + bz), p_t = x_t @ Wh^T + bh,
  g(x) = x + 0.5 for x>=0 else sigmoid(x)  (note g(x) = max(x+0.5, sigmoid(x))),
  initial state g(h_0).  Shapes: x [4, 4096, 1024], H = 1024.

Sharding: 8 cores = batch(4) x H-halves(2). No collectives. Each core gets
host-pre-transposed inputs (xT bf16/fp8 [din, seq], pre-interleaved fp8
weights for DoubleRowSwInterleave, bf16 weights x32 for the bf16 k-tiles)
and returns hT [512 ch, 4096 seq] bf16; host transposes/upcasts back.

Precision plan (gate 2e-2; measured 1.866e-2 at kh_fp8_pairs=3,
bit-deterministic across runs/processes):
  - gate GEMM kz: full fp8-e4m3 DoubleRowSwInterleave (sigmoid squashes err)
  - candidate GEMM kh: k-tiles [0, 2p) in fp8-DRSW, k-tiles [2p, 8) in bf16,
    accumulating into one PSUM tile. All weights pre-scaled x32 so fp8 W
    lands in e4m3 range; activations apply scale 1/32 when reading PSUM.
    (x quantization error dominates; GPTQ on W was tested and gains ~nothing.)
  - consumer chain + h store in bf16 (scan keeps fp32 internal state);
    host upcasts the bf16 output back to f32.

Device dataflow per (seq-block of 1024, chan-group of 128):
  PE:  kz: 8 DRSW fp8 matmuls -> PSUM 32*kz
       kh: 2*(p DRSW + (8-2p) bf16) matmuls -> PSUM 32*kh
  ACT:  z = sigmoid(kz/32 + bz), sp = sigmoid(kh/32 + bh)     (bf16 out)
  POOL: a = 1 - z                                             (bf16)
  DVE:  lin = kh/32 + (bh+0.5); gp = max(lin, sp); b = z*gp;
        h = scan(a, b, init)            (bf16 ops, scan state fp32)
DMA: sync ring = loads, scalar ring = stores. x loads are one descriptor
per (tensor, block) to minimize PE-queue semaphore waits; block 0 is split
finer (and emits kz(m0,m1) first) so the first matmul chains start early.

Measured (8-core SPMD, For_i slope method): ~91-96 us vs 116-119 us for the
prior all-bf16-kh version. GEMM stage alone (stages="mmonly") is ~78 us —
the kernel is PE-bound; ACT/DVE/Pool/DMA all hide behind it except a few
us of drain at the tail. Engine notes baked into this design: GPSIMD (Pool)
cannot read PSUM or run TensorTensor/scan opcodes; DMA cannot write PSUM;
matmul N is capped at 512 by the f32 PSUM bank; DoubleRow is fp8-only.
"""

import numpy as np
import ml_dtypes

import concourse.bass as bass
import concourse.bacc as bacc
import concourse.mybir as mybir
import concourse.tile as tile
from concourse.bass_utils import run_bass_kernel_spmd

F32 = mybir.dt.float32
BF16 = mybir.dt.bfloat16
F8 = mybir.dt.float8e4
AF = mybir.ActivationFunctionType
ALU = mybir.AluOpType
DRSW = mybir.MatmulPerfMode.DoubleRowSwInterleave

BS, SEQ, DIN, H = 4, 4096, 1024, 1024
NCORES = 8
H_SPLIT = 2
CH = H // H_SPLIT  # channels per core
W_SCALE = 32.0  # all weights pre-scaled x32; consumers apply 1/32
DEFAULT_P = 3   # kh k-tile pairs in fp8 (0..4); 3 => 6/8 of the candidate GEMM
                # (measured rel err 1.866e-2 vs the 2e-2 gate, bit-deterministic)


def build_nc(seq=SEQ, din=DIN, ch=CH, nb=1024, loop_reps=1, x_bufs=3,
             epool_bufs=2, h_bufs=2, kz_bufs=2, kh_bufs=2, stages="full",
             mm_paths="both", blk0_scalar=False, tail_split=True,
             kh_fp8_pairs=DEFAULT_P, dr_free=512, tail_blocks=0,
             scan_engine="vector", lin_engine="vector"):
    """Build the single-core SPMD Bass program.

    loop_reps > 1 wraps the body in a hardware For_i loop purely for
    benchmarking (wall-clock slope between two loop_reps isolates HW exec
    time from RPC overhead).
    """
    kt = din // 128        # contraction tiles
    mg = ch // 128         # chan groups
    nblk = seq // nb       # seq blocks
    p = kh_fp8_pairs
    kbf = kt - 2 * p       # bf16 k-tiles for kh
    nmm = nb // 512        # bf16 MM free-dim chunks
    ndr = nb // dr_free    # DR free-dim chunks

    nc = bacc.Bacc("TRN2", target_bir_lowering=False, debug=False)

    x16_d = nc.dram_tensor("xT16", [din, seq], BF16, kind="ExternalInput")
    x8_d = nc.dram_tensor("xT8", [din, seq], F8, kind="ExternalInput")
    wzi_d = nc.dram_tensor("wzi", [128, kt // 2, mg, 256], F8, kind="ExternalInput")
    aux_d = nc.dram_tensor("aux", [128, 5, mg], F32, kind="ExternalInput")
    auxb_d = nc.dram_tensor("auxb", [128, 1, mg], BF16, kind="ExternalInput")
    if p > 0:
        whi_d = nc.dram_tensor("whi", [128, p, mg, 256], F8, kind="ExternalInput")
    if kbf > 0:
        whb_d = nc.dram_tensor("whb", [128, kbf, ch], BF16, kind="ExternalInput")
    hT_d = nc.dram_tensor("hT", [ch, seq], BF16, kind="ExternalOutput")

    x16_r = x16_d.ap().rearrange("(k p) s -> p k s", p=128)
    x8_r = x8_d.ap().rearrange("(k p) s -> p k s", p=128)

    with tile.TileContext(nc) as tc:
        with (
            tc.tile_pool(name="wpool", bufs=1) as wpool,
            tc.tile_pool(name="xpool", bufs=x_bufs) as xpool,
            tc.tile_pool(name="epool", bufs=epool_bufs) as epool,
            tc.tile_pool(name="hpool", bufs=1) as hpool,
            tc.tile_pool(name="psum", bufs=1, space="PSUM") as psum,
        ):
            aux_sb = wpool.tile([128, 5, mg], F32)
            auxb_sb = wpool.tile([128, 1, mg], BF16)
            wzi_sb = wpool.tile([128, kt // 2, mg, 256], F8)
            if p > 0:
                whi_sb = wpool.tile([128, p, mg, 256], F8)
            if kbf > 0:
                whb_sb = wpool.tile([128, kbf, ch], BF16)

            nc.sync.dma_start(aux_sb[:], aux_d.ap())
            nc.sync.dma_start(auxb_sb[:], auxb_d.ap())
            # wzi split per kp: the first kz chain only waits on its own slice
            for kp in range(kt // 2):
                nc.sync.dma_start(wzi_sb[:, kp, :, :], wzi_d.ap()[:, kp, :, :])

            # block schedule: (offset, width); tail_blocks=2 splits the last
            # 1024-block into two 512 halves so the consumer drain is shorter
            if tail_blocks:
                blocks = [(i * nb, nb) for i in range(nblk - 1)]
                w = nb // tail_blocks
                blocks += [(seq - nb + i * w, w) for i in range(tail_blocks)]
            else:
                blocks = [(i * nb, nb) for i in range(nblk)]

            def emit_body():
                h_prev = [auxb_sb[:, 0, m : m + 1] for m in range(mg)]
                for bi, (off, nbb) in enumerate(blocks):
                    blk = bi
                    sl = slice(off, off + nbb)
                    nbs = slice(0, nbb)
                    xb8 = xpool.tile([128, kt, nb], F8, tag="xb8", name="xb8")
                    if kbf > 0 and mm_paths != "kz":
                        xb16 = xpool.tile([128, kbf, nb], BF16, tag="xb16", name="xb16")
                    if blk == 0:
                        # startup: fine-grained so the first MM chains start
                        # as soon as their slices land; kh weights ride
                        # between the xb8 halves so kh(m0) isn't starved
                        if mm_paths != "kh":
                            nc.sync.dma_start(xb8[:, 0 : kt // 2, :], x8_r[:, 0 : kt // 2, sl])
                        ring0 = nc.scalar if blk0_scalar else nc.sync
                        if p > 0 and mm_paths != "kz":
                            ring0.dma_start(whi_sb[:], whi_d.ap())
                        if mm_paths != "kh":
                            nc.sync.dma_start(xb8[:, kt // 2 : kt, :], x8_r[:, kt // 2 : kt, sl])
                        if kbf > 0 and mm_paths != "kz":
                            ring0.dma_start(whb_sb[:], whb_d.ap())
                            h1 = max(1, kbf // 2)
                            ring0.dma_start(xb16[:, 0:h1, :], x16_r[:, 2 * p : 2 * p + h1, sl])
                            if kbf > h1:
                                ring0.dma_start(xb16[:, h1:kbf, :], x16_r[:, 2 * p + h1 : kt, sl])
                    else:
                        if mm_paths != "kh":
                            nc.sync.dma_start(xb8[:, :, nbs], x8_r[:, :, sl])
                        if kbf > 0 and mm_paths != "kz":
                            nc.sync.dma_start(xb16[:, :, nbs], x16_r[:, 2 * p : kt, sl])

                    def emit_kz(m):
                        kz = psum.tile([128, nb], F32, tag="kz", bufs=kz_bufs, name="kz")
                        for j in range(max(1, nbb // dr_free)):
                            js = slice(j * dr_free, min((j + 1) * dr_free, nbb))
                            for kp in range(kt // 2):
                                nc.tensor.matmul(
                                    kz[:, js], wzi_sb[:, kp, m, :],
                                    xb8[:, 2 * kp : 2 * kp + 2, js],
                                    start=(kp == 0), stop=(kp == kt // 2 - 1),
                                    perf_mode=DRSW,
                                )
                        return kz

                    def emit_kh(m):
                        ms = slice(m * 128, (m + 1) * 128)
                        kh = psum.tile([128, nb], F32, tag="kh", bufs=kh_bufs, name="kh")
                        for j in range(max(1, nbb // 512)):
                            js = slice(j * 512, min((j + 1) * 512, nbb))
                            for kp in range(p):
                                nc.tensor.matmul(
                                    kh[:, js], whi_sb[:, kp, m, :],
                                    xb8[:, 2 * kp : 2 * kp + 2, js],
                                    start=(kp == 0),
                                    stop=(p == kt // 2 and kp == p - 1),
                                    perf_mode=DRSW,
                                    skip_group_check=True,
                                )
                            for k in range(kbf):
                                nc.tensor.matmul(
                                    kh[:, js], whb_sb[:, k, ms], xb16[:, k, js],
                                    start=(p == 0 and k == 0),
                                    stop=(k == kbf - 1),
                                    skip_group_check=True,
                                )
                        return kh

                    if mm_paths == "kz":
                        for m in range(mg):
                            emit_kz(m)
                        continue
                    if mm_paths == "kh":
                        for m in range(mg):
                            emit_kh(m)
                        continue

                    if blk == 0:
                        # kz(m0, m1) run while kh weights/xb16 still load;
                        # paired order would stall kh(m0) on those DMAs
                        kzs = {i: emit_kz(i) for i in range(2)}
                    for m in range(mg):
                        ms = slice(m * 128, (m + 1) * 128)
                        kz = kzs.pop(m) if blk == 0 and m in kzs else emit_kz(m)
                        kh = emit_kh(m)
                        if stages == "mmonly":
                            continue

                        # Last group: half-granular consumers so the drain
                        # overlaps the final matmul chains.
                        last = tail_split and bi == len(blocks) - 1 and m == mg - 1
                        halves = (
                            [slice(0, nbb // 2), slice(nbb // 2, nbb)] if last
                            else [slice(0, nbb)]
                        )
                        h_t = hpool.tile([128, nb], BF16, tag=f"h{m}", bufs=h_bufs, name="h_t")
                        for hs in halves:
                            w = hs.stop - hs.start
                            tg = str(w)
                            z_t = epool.tile([128, w], BF16, tag="z" + tg, name="z_t")
                            a_t = epool.tile([128, w], BF16, tag="a" + tg, name="a_t")
                            sp_t = epool.tile([128, w], BF16, tag="sp" + tg, name="sp_t")
                            lin_t = epool.tile([128, w], BF16, tag="lin" + tg, name="lin_t")

                            # z = sigmoid(kz/32 + bz) on ACT
                            nc.scalar.activation(
                                z_t[:], kz[:, hs], AF.Sigmoid,
                                bias=aux_sb[:, 1, m : m + 1], scale=1.0 / W_SCALE,
                            )
                            # a = 1 - z (Pool unless it owns the scan)
                            a_eng = nc.vector if scan_engine == "gpsimd" else nc.gpsimd
                            a_eng.tensor_scalar(
                                a_t[:], z_t[:], -1.0, 1.0, op0=ALU.mult, op1=ALU.add,
                            )
                            # sp = sigmoid(kh/32 + bh) on ACT
                            nc.scalar.activation(
                                sp_t[:], kh[:, hs], AF.Sigmoid,
                                bias=aux_sb[:, 3, m : m + 1], scale=1.0 / W_SCALE,
                            )
                            # lin = kh/32 + (bh + 0.5)
                            if lin_engine == "vector":
                                nc.vector.tensor_scalar(
                                    lin_t[:], kh[:, hs], 1.0 / W_SCALE,
                                    aux_sb[:, 4, m : m + 1],
                                    op0=ALU.mult, op1=ALU.add,
                                )
                            else:
                                nc.scalar.activation(
                                    lin_t[:], kh[:, hs], AF.Identity,
                                    bias=aux_sb[:, 4, m : m + 1], scale=1.0 / W_SCALE,
                                )
                            if stages == "act":
                                continue
                            gp_t = epool.tile([128, w], BF16, tag="gp" + tg, name="gp_t")
                            b_t = epool.tile([128, w], BF16, tag="b" + tg, name="b_t")
                            # gp = max(lin, sp) on DVE (bf16 2x)
                            nc.vector.tensor_max(gp_t[:], lin_t[:], sp_t[:])
                            nc.vector.tensor_mul(b_t[:], z_t[:], gp_t[:])
                            # h scan: state = a*state + b (fp32 internal state)
                            if scan_engine == "copy":  # timing probe only
                                nc.vector.tensor_copy(h_t[:, hs], b_t[:])
                            else:
                                nc.vector.tensor_tensor_scan(
                                    h_t[:, hs], a_t[:], b_t[:], h_prev[m],
                                    op0=ALU.mult, op1=ALU.add,
                                )
                            h_prev[m] = h_t[:, hs.stop - 1 : hs.stop]

                            if stages == "full":
                                # stores on their own ring so a store's sem
                                # wait can't block load prefetches
                                nc.scalar.dma_start(
                                    hT_d.ap()[ms, off + hs.start : off + hs.stop],
                                    h_t[:, hs],
                                )

            if loop_reps == 1:
                emit_body()
            else:
                with tc.For_i(0, loop_reps, 1):
                    emit_body()

    nc.compile()
    return nc


def _g(x):
    return np.where(x >= 0, x + 0.5, 1.0 / (1.0 + np.exp(-x)))


def _interleave(w8, din, mg):
    """Host DRSW pre-interleave: flat[2c+i] = W_i[:, 127-c] per (kp, m)."""
    kt = din // 128
    a = w8.reshape(kt, 128, mg, 128)         # [k, p, m, c]
    a = a.transpose(1, 0, 2, 3)              # [p, k, m, c]
    a = a.reshape(128, kt // 2, 2, mg, 128)  # [p, kp, i, m, c]
    a = a[..., ::-1]                         # reverse c
    a = a.transpose(0, 1, 3, 4, 2)           # [p, kp, m, c_rev, i]
    return np.ascontiguousarray(a.reshape(128, kt // 2, mg, 256))


def make_in_maps(x, h_0, Wz, bz, Wh, bh, seq=SEQ, din=DIN, ch=CH,
                 kh_fp8_pairs=DEFAULT_P):
    """Host-side shard: returns one in_map per core."""
    mg = ch // 128
    kt = din // 128
    p = kh_fp8_pairs
    kbf = kt - 2 * p
    gh0 = _g(h_0.astype(np.float32))  # [bs, 1, H]
    xT16 = [np.ascontiguousarray(x[b].T).astype(ml_dtypes.bfloat16) for b in range(BS)]
    xT8 = [t.astype(ml_dtypes.float8_e4m3) for t in xT16]
    in_maps = []
    for c in range(NCORES):
        b, grp = divmod(c, H_SPLIT)
        cs = slice(grp * ch, (grp + 1) * ch)
        wz8 = np.ascontiguousarray(
            (Wz[cs, :] * W_SCALE).T
        ).astype(ml_dtypes.float8_e4m3)
        wh32T = np.ascontiguousarray((Wh[cs, :] * W_SCALE).T)  # [din, ch]
        wh8 = wh32T.astype(ml_dtypes.float8_e4m3)
        whi_full = _interleave(wh8, din, mg)
        # bf16 k-tiles as [128, kbf, ch]
        whb = np.ascontiguousarray(
            wh32T[2 * p * 128 :, :].reshape(kbf, 128, ch).transpose(1, 0, 2)
        ).astype(ml_dtypes.bfloat16) if kbf > 0 else None
        aux = np.zeros((128, 5, mg), dtype=np.float32)
        aux[:, 0, :] = gh0[b, 0, cs].reshape(mg, 128).T
        aux[:, 1, :] = bz[cs].reshape(mg, 128).T
        aux[:, 2, :] = (bh[cs] - 0.5).reshape(mg, 128).T  # psum_bias mode sp bias
        aux[:, 3, :] = bh[cs].reshape(mg, 128).T
        aux[:, 4, :] = (bh[cs] + 0.5).reshape(mg, 128).T
        auxb = gh0[b, 0, cs].reshape(mg, 128).T.astype(ml_dtypes.bfloat16)
        im = {
            "xT16": xT16[b],
            "xT8": xT8[b],
            "wzi": _interleave(wz8, din, mg),
            "aux": aux,
            "auxb": np.ascontiguousarray(auxb[:, None, :]),
        }
        if p > 0:
            im["whi"] = np.ascontiguousarray(whi_full[:, :p])
        if kbf > 0:
            im["whb"] = whb
        in_maps.append(im)
    return in_maps


_NC_CACHE = {}


def get_nc():
    if "nc" not in _NC_CACHE:
        _NC_CACHE["nc"] = build_nc()
    return _NC_CACHE["nc"]


def kernel(x, h_0, Wz, bz, Wh, bh, trace=False, trace_kwargs=None):
    x = np.asarray(x)
    h_0 = np.asarray(h_0)
    Wz = np.asarray(Wz)
    bz = np.asarray(bz)
    Wh = np.asarray(Wh)
    bh = np.asarray(bh)

    nc = get_nc()
    in_maps = make_in_maps(x, h_0, Wz, bz, Wh, bh)
    # run twice: the very first execution on a fresh device context has been
    # observed (rarely) to return garbage; the repeat costs ~100ms
    run_bass_kernel_spmd(nc, in_maps, core_ids=list(range(NCORES)))
    res = run_bass_kernel_spmd(
        nc, in_maps, core_ids=list(range(NCORES)),
        trace=trace, **(trace_kwargs or {}),
    )
    out = np.empty((BS, SEQ, H), dtype=np.float32)
    for c in range(NCORES):
        b, grp = divmod(c, H_SPLIT)
        out[b, :, grp * CH : (grp + 1) * CH] = res.results[c]["hT"].astype(np.float32).T
    if trace:
        kernel.last_result = res
    return out
